# revision 2
# baseline (speedup 1.0000x reference)
"""Trainium2 Bass kernel for nn_ContrastiveLossV2 (8-core SPMD).

Reference computation:
    z = l2norm(concat([emb_i, emb_j]))          # [8192, 128]
    sim = z @ z.T                               # [8192, 8192]
    loss = mean((sim - class_pairs)**2)

Instead of materializing sim and running an elementwise (sim-cp)^2 pass
(8.4M elements/core on the vector/scalar engines — slower than the DMA
roofline), expand the square:

    sum((sim-cp)^2) = sum(sim^2) - 2*sum(sim*cp) + sum(cp^2)

  * sum(sim^2)  = ||Z^T Z||_F^2   (Gram trick; per-core G_c = Z_c^T Z_c over
                  its 1024 local rows, G = sum_c G_c summed on host in f64)
  * sum(sim*cp) = sum_{d,c} V[d,c] * zT[d,c]  with  V = Z_loc^T @ CP_loc —
                  computed by the TensorEngine directly from *row-major* cp
                  tiles (contraction over the local row dim), so cp never
                  needs a transpose.
  * sum(cp^2)   = one Square+accumulate pass on the scalar engine.

Row sharding: core c owns reps rows [c*1024, (c+1)*1024) (cores 0-3 from
emb_i, 4-7 from emb_j) and the matching 1024-row slice of class_pairs.
Everything per-core-specific arrives as data (emb_loc / cp_loc inputs), so
one SPMD program serves all 8 cores. class_pairs is cast f32->bf16 in-flight
by the SWDGE DMA; all matmuls run in bf16 with f32 PSUM accumulation.
Per-core partial sums return as small f32 tensors; the host combines them in
f64. The kernel is DMA-bound: 32MB of class_pairs per core at ~358 GB/s.
"""

import numpy as np

import concourse.bacc as bacc
import concourse.mybir as mybir
import concourse.tile as tile
from concourse.bass_utils import run_bass_kernel_spmd

f32 = mybir.dt.float32
bf16 = mybir.dt.bfloat16
AF = mybir.ActivationFunctionType
OP = mybir.AluOpType

N_CORES = 8
N, D = 4096, 128
TWO_N = 2 * N                     # 8192
R_LOC = TWO_N // N_CORES          # 1024 rows per core
M_BLK = R_LOC // 128              # 8 local 128-row blocks
NCH = 512                         # matmul free-dim chunk (one PSUM bank)
N4 = 2048                         # cp DMA tile width (1MB f32 reads)
N_GRP = TWO_N // N4               # 4 column groups
K_PER_GRP = N4 // NCH             # 4 chunks per group
EPS = 1e-12

_cached = {}


def _build_module():
    nc = bacc.Bacc("TRN2", target_bir_lowering=False, debug=False,
                   num_devices=N_CORES)

    emb_i = nc.dram_tensor("emb_i", [N, D], f32, kind="ExternalInput")
    emb_j = nc.dram_tensor("emb_j", [N, D], f32, kind="ExternalInput")
    emb_loc = nc.dram_tensor("emb_loc", [R_LOC, D], f32, kind="ExternalInput")
    cp_loc = nc.dram_tensor("cp_loc", [R_LOC, TWO_N], f32, kind="ExternalInput")
    ident = nc.dram_tensor("ident", [128, 128], bf16, kind="ExternalInput")

    out_g = nc.dram_tensor("out_g", [128, 128], f32, kind="ExternalOutput")
    out_cp = nc.dram_tensor("out_cp", [128, M_BLK * N_GRP], f32,
                            kind="ExternalOutput")
    out_x = nc.dram_tensor("out_x", [128, N_GRP * K_PER_GRP], f32,
                           kind="ExternalOutput")

    with tile.TileContext(nc) as tc:
        with (
            tc.tile_pool(name="const", bufs=1) as const_pool,
            tc.tile_pool(name="persist", bufs=1) as persist,
            tc.tile_pool(name="stag", bufs=2) as stag_pool,
            tc.tile_pool(name="sq", bufs=2) as sq_pool,
            tc.tile_pool(name="norm", bufs=4) as norm_pool,
            tc.tile_pool(name="zc", bufs=4) as zc_pool,
            tc.tile_pool(name="cpt", bufs=14) as cp_pool,
            tc.tile_pool(name="tmp", bufs=4) as tmp_pool,
            tc.tile_pool(name="sqj", bufs=2) as sqj_pool,
            tc.tile_pool(name="psv", bufs=4, space="PSUM") as psv_pool,
            tc.tile_pool(name="pst", bufs=2, space="PSUM") as pst_pool,
            tc.tile_pool(name="psg", bufs=1, space="PSUM") as psg_pool,
        ):
            ident_sb = const_pool.tile([128, 128], bf16)
            nc.sync.dma_start(out=ident_sb[:], in_=ident[:])

            # zT[d, r] = normalized reps row r, transposed. bf16.
            zT = persist.tile([128, TWO_N], bf16)
            # local row-major z tiles (natural 128-row blocks), bf16.
            z_loc = persist.tile([128, M_BLK, 128], bf16)
            # accumulators for the partial sums
            acc_cp = persist.tile([128, M_BLK * N_GRP], f32)
            acc_x = persist.tile([128, N_GRP * K_PER_GRP], f32)

            def normalize(stag, n_chunks, emit_chunk):
                """stag: [128, n_chunks, 128] f32 staging (one emb row per
                (partition, chunk)). Calls emit_chunk(n, zc) with the
                normalized bf16 [128,128] chunk."""
                sq = sq_pool.tile([128, n_chunks, 128], f32, tag="sq")
                nc.scalar.activation(sq[:], stag[:], AF.Square)
                nsq = norm_pool.tile([128, n_chunks], f32, tag="nsq")
                nc.vector.tensor_reduce(nsq[:], sq[:], axis=mybir.AxisListType.X,
                                        op=OP.add)
                nrm = norm_pool.tile([128, n_chunks], f32, tag="nrm")
                nc.scalar.activation(nrm[:], nsq[:], AF.Sqrt)
                nc.vector.tensor_scalar_max(nrm[:], nrm[:], EPS)
                rec = norm_pool.tile([128, n_chunks], f32, tag="rec")
                nc.vector.reciprocal(rec[:], nrm[:])
                for n in range(n_chunks):
                    zc = zc_pool.tile([128, 128], bf16, tag="zc")
                    nc.vector.tensor_scalar_mul(zc[:], stag[:, n, :],
                                                rec[:, n:n + 1])
                    emit_chunk(n, zc)

            # ---- phase A: build zT from emb_i / emb_j ----
            for ei, emb in enumerate((emb_i, emb_j)):
                stag = stag_pool.tile([128, 32, 128], f32, tag="stag")
                # (p n) d -> p n d: partition p, chunk n holds row 32p+n;
                # 16KB contiguous per partition -> max DMA efficiency.
                nc.sync.dma_start(out=stag[:],
                                  in_=emb[:].rearrange("(p n) d -> p n d", p=128))

                base = ei * N
                ps4 = [None]

                def emit(n, zc, base=base, ps4=ps4):
                    g, dlt = divmod(n, 4)
                    if dlt == 0:
                        ps4[0] = pst_pool.tile([128, 4, 128], bf16, tag="ps4",
                                               name="ps4")
                    nc.tensor.transpose(ps4[0][:, dlt, :], zc[:], ident_sb[:])
                    if dlt == 3:
                        # reps row of (chunk n, partition p) is base+32p+n, so
                        # chunks 4g..4g+3 scatter to zT columns base+4g+32p+dlt.
                        dst = zT[:, base:base + N] \
                            .rearrange("q (p n) -> q n p", n=32)[:, 4 * g:4 * g + 4, :]
                        nc.vector.tensor_copy(dst, ps4[0][:])

                normalize(stag, 32, emit)

            # ---- phase B: local row blocks (natural order) ----
            stag_l = stag_pool.tile([128, M_BLK, 128], f32, tag="stag_loc")
            # (n p) d -> p n d: partition p, chunk n holds row n*128+p, i.e.
            # chunk n is the natural 128-row block n (matches cp row order).
            nc.sync.dma_start(out=stag_l[:],
                              in_=emb_loc[:].rearrange("(n p) d -> p n d", p=128))
            normalize(stag_l, M_BLK,
                      lambda n, zc: nc.vector.tensor_copy(z_loc[:, n, :], zc[:]))

            # ---- phase C: G = Z_loc^T @ Z_loc (local Gram, 128x128) ----
            g_ps = psg_pool.tile([128, 128], f32)
            for m in range(M_BLK):
                nc.tensor.matmul(g_ps[:], lhsT=z_loc[:, m, :], rhs=z_loc[:, m, :],
                                 start=(m == 0), stop=(m == M_BLK - 1))
            g_sb = tmp_pool.tile([128, 128], f32, tag="gsb")
            nc.scalar.copy(g_sb[:], g_ps[:])
            nc.sync.dma_start(out=out_g[:], in_=g_sb[:])

            # ---- phase D: stream class_pairs ----
            for n4 in range(N_GRP):
                cpts = []
                for m in range(M_BLK):
                    cpt = cp_pool.tile([128, N4], bf16, tag="cpt")
                    # SWDGE cast f32 -> bf16 in flight
                    nc.gpsimd.dma_start(
                        out=cpt[:],
                        in_=cp_loc[m * 128:(m + 1) * 128,
                                   n4 * N4:(n4 + 1) * N4])
                    cpts.append(cpt)
                for k in range(K_PER_GRP):
                    ps = psv_pool.tile([128, NCH], f32, tag="psv")
                    for m in range(M_BLK):
                        nc.tensor.matmul(ps[:], lhsT=z_loc[:, m, :],
                                         rhs=cpts[m][:, k * NCH:(k + 1) * NCH],
                                         start=(m == 0), stop=(m == M_BLK - 1))
                    col0 = n4 * N4 + k * NCH
                    tmp = tmp_pool.tile([128, NCH], f32, tag="xtmp")
                    nc.vector.tensor_tensor(tmp[:], ps[:], zT[:, col0:col0 + NCH],
                                            op=OP.mult)
                    nc.vector.tensor_reduce(acc_x[:, n4 * K_PER_GRP + k:
                                                  n4 * K_PER_GRP + k + 1],
                                            tmp[:], axis=mybir.AxisListType.X,
                                            op=OP.add)
                for m in range(M_BLK):
                    sqj = sqj_pool.tile([128, N4], bf16, tag="sqj")
                    nc.scalar.activation(sqj[:], cpts[m][:], AF.Square,
                                         accum_out=acc_cp[:, n4 * M_BLK + m:
                                                          n4 * M_BLK + m + 1])

            nc.sync.dma_start(out=out_cp[:], in_=acc_cp[:])
            nc.sync.dma_start(out=out_x[:], in_=acc_x[:])

    nc.compile()
    return nc


def _get_module():
    if "nc" not in _cached:
        _cached["nc"] = _build_module()
    return _cached["nc"]


def kernel(emb_i, emb_j, class_pairs, _return_raw=False, _trace=False):
    import ml_dtypes

    emb_i = np.ascontiguousarray(emb_i, dtype=np.float32)
    emb_j = np.ascontiguousarray(emb_j, dtype=np.float32)
    class_pairs = np.ascontiguousarray(class_pairs, dtype=np.float32)
    ident = np.eye(128, dtype=ml_dtypes.bfloat16)

    nc = _get_module()
    in_maps = []
    for c in range(N_CORES):
        r0 = c * R_LOC
        if r0 < N:
            emb_loc = emb_i[r0:r0 + R_LOC]
        else:
            emb_loc = emb_j[r0 - N:r0 - N + R_LOC]
        in_maps.append({
            "emb_i": emb_i,
            "emb_j": emb_j,
            "emb_loc": np.ascontiguousarray(emb_loc),
            "cp_loc": np.ascontiguousarray(class_pairs[r0:r0 + R_LOC]),
            "ident": ident,
        })

    res = run_bass_kernel_spmd(nc, in_maps, list(range(N_CORES)), trace=_trace)

    G = np.zeros((128, 128), dtype=np.float64)
    sum_cp2 = 0.0
    cross = 0.0
    for c in range(N_CORES):
        G += res.results[c]["out_g"].astype(np.float64)
        sum_cp2 += res.results[c]["out_cp"].astype(np.float64).sum()
        cross += res.results[c]["out_x"].astype(np.float64).sum()
    sum_sim2 = float((G * G).sum())
    loss = (sum_sim2 - 2.0 * cross + sum_cp2) / float(TWO_N * TWO_N)
    out = np.asarray(loss, dtype=np.float32)
    if _return_raw:
        return out, res
    return out
